# revision 1
# baseline (speedup 1.0000x reference)
"""Trainium2 Bass kernel for nn_NeuralNet_19516331393457 (dense_mlp).

Pipeline: x = embed[data] (48-entry table); h1 = relu(x@W1+b1);
h2 = tanh(h1@W2+b2); out = h2@W3+b3; return out[argmax(F(out0, out1))].

Strategy (data-parallel over N=500000 on 8 cores):
  - Host: tiny-table gather embed[data] fused with a tile-blocked transpose
    so each device chunk is a contiguous [128 feat, 512 samples] tile.
  - Device (per core, 62976 padded samples = 123 chunks of 512), float32r
    matmuls (measured ~1 cyc/col when batched with the same stationary):
      * chunks grouped by 4 so each weight matrix stays stationary across
        4 back-to-back matmuls (LDWEIGHTS amortized)
      * MM3 [2,512] outputs pair-packed at free offsets of a [2,1024]
        two-bank PSUM tile; one pack-copy per 2 chunks into [2, 16K]
        staging pieces; 4 contiguous output DMAs per core
      * PSUM evictions (relu, tanh, pack-copy) balanced across DVE/ACT
  - Host: decode [2, NPC] outs, compute F in float64, global argmax,
    return out[argmax] + b3 (min-subtraction doesn't change argmax).
"""

import numpy as np

import concourse.mybir as mybir
import concourse.tile as tile
from concourse import bacc
from concourse.bass_utils import run_bass_kernel_spmd

N = 500000
D = 128
H1 = 128
H2 = 64
NCLS = 2
NCORES = 8
CHUNK = 512
NPC_RAW = N // NCORES              # 62500 samples per core
CHUNKS = -(-NPC_RAW // CHUNK)      # 123 chunks per core
NPC = CHUNKS * CHUNK               # 62976 padded samples per core
G = 4                              # chunks per stationary-weight group
QC = 32                            # chunks per output staging piece

_F32 = mybir.dt.float32
_F32R = mybir.dt.float32r


def _build_bass():
    nc = bacc.Bacc(
        "TRN2",
        target_bir_lowering=False,
        debug=False,
        enable_asserts=False,
        num_devices=NCORES,
    )
    # x is stored two chunks per DMA tile: [ceil(CHUNKS/2), D, 2*CHUNK]
    npairs = (CHUNKS + 1) // 2
    x_t = nc.dram_tensor("x_t", [npairs, D, 2 * CHUNK], _F32R,
                         kind="ExternalInput")
    w1 = nc.dram_tensor("w1", [D, H1], _F32R, kind="ExternalInput")
    w2 = nc.dram_tensor("w2", [H1, H2], _F32R, kind="ExternalInput")
    w3 = nc.dram_tensor("w3", [H2, NCLS], _F32R, kind="ExternalInput")
    b1 = nc.dram_tensor("b1", [H1, 1], _F32, kind="ExternalInput")
    b2 = nc.dram_tensor("b2", [H2, 1], _F32, kind="ExternalInput")
    out_d = nc.dram_tensor("out_pairs", [2, NPC], _F32, kind="ExternalOutput")

    with tile.TileContext(nc) as tc:
        with (
            tc.tile_pool(name="w", bufs=1) as wpool,
            tc.tile_pool(name="x", bufs=3) as xpool,
            tc.tile_pool(name="h1", bufs=6) as h1pool,
            tc.tile_pool(name="h2", bufs=4) as h2pool,
            tc.tile_pool(name="ob", bufs=2) as obpool,
            tc.tile_pool(name="p1", bufs=4, space="PSUM") as p1pool,
            tc.tile_pool(name="p2", bufs=2, space="PSUM") as p2pool,
            tc.tile_pool(name="po", bufs=2, space="PSUM") as popool,
        ):
            w1sb = wpool.tile([D, H1], _F32R)
            nc.sync.dma_start(w1sb[:], w1[:, :])
            w2sb = wpool.tile([H1, H2], _F32R)
            nc.sync.dma_start(w2sb[:], w2[:, :])
            w3sb = wpool.tile([H2, NCLS], _F32R)
            nc.sync.dma_start(w3sb[:], w3[:, :])
            b1sb = wpool.tile([H1, 1], _F32)
            nc.sync.dma_start(b1sb[:], b1[:, :])
            b2sb = wpool.tile([H2, 1], _F32)
            nc.sync.dma_start(b2sb[:], b2[:, :])

            xts = {}     # pair index -> sbuf tile [D, 2*CHUNK]
            h1ts = {}    # chunk -> sbuf tile [H1, CHUNK]
            h2ts = {}    # chunk -> sbuf tile [H2, CHUNK]
            pos = {}     # pair index -> psum tile [2, 2*CHUNK]
            obs = {}     # piece index -> sbuf tile [2, QC*CHUNK]
            p1s = {}

            ngroups = -(-CHUNKS // G)
            for g in range(ngroups):
                chunks = list(range(G * g, min(G * (g + 1), CHUNKS)))

                for c in chunks:
                    if c % 2 == 0:
                        xt = xpool.tile([D, 2 * CHUNK], _F32R,
                                        name=f"xt{c // 2}", tag="xt")
                        nc.sync.dma_start(xt[:], x_t[c // 2, :, :])
                        xts[c // 2] = xt

                # MM1 x4 (W1 stationary)
                for c in chunks:
                    p1 = p1pool.tile([H1, CHUNK], _F32, name=f"p1_{c}",
                                     tag="p1")
                    xsrc = xts[c // 2][:, (c % 2) * CHUNK : (c % 2 + 1) * CHUNK]
                    nc.tensor.matmul(p1[:], w1sb[:], xsrc, start=True,
                                     stop=True)
                    p1s[c] = p1
                # relu evictions: h1 = max(p1 + b1, 0); alternate DVE/ACT
                for c in chunks:
                    h1t = h1pool.tile([H1, CHUNK], _F32R, name=f"h1_{c}",
                                      tag="h1")
                    nc.vector.tensor_scalar(
                        h1t[:], p1s[c][:], b1sb[:], 0.0,
                        mybir.AluOpType.add, mybir.AluOpType.max,
                    )
                    h1ts[c] = h1t

                # MM2 x4 (W2 stationary)
                for c in chunks:
                    p2 = p2pool.tile([H2, CHUNK], _F32, name=f"p2_{c}",
                                     tag="p2")
                    nc.tensor.matmul(p2[:], w2sb[:], h1ts[c][:], start=True,
                                     stop=True)
                    h2t = h2pool.tile([H2, CHUNK], _F32R, name=f"h2_{c}",
                                      tag="h2")
                    nc.scalar.activation(
                        h2t[:], p2[:], mybir.ActivationFunctionType.Tanh,
                        bias=b2sb[:],
                    )
                    h2ts[c] = h2t

                # MM3 x4 (W3 stationary) -> [2, CHUNK] psum, packed into
                # [2, QC*CHUNK] staging pieces, 4 output DMAs per core
                for c in chunks:
                    po = popool.tile([2, CHUNK], _F32, name=f"po_{c}",
                                     tag="po")
                    nc.tensor.matmul(po[:], w3sb[:], h2ts[c][:], start=True,
                                     stop=True)
                    q = c // QC
                    if q not in obs:
                        obs[q] = obpool.tile([2, QC * CHUNK], _F32,
                                             name=f"ob{q}", tag="ob")
                    dst = obs[q][:, (c % QC) * CHUNK : (c % QC + 1) * CHUNK]
                    if c % 2 == 0:
                        nc.vector.tensor_copy(dst, po[:])
                    else:
                        nc.scalar.copy(dst, po[:])
                    if c == CHUNKS - 1 or (c % QC) == QC - 1:
                        npiece = min(QC * CHUNK, NPC - q * QC * CHUNK)
                        nc.sync.dma_start(
                            out_d[:, q * QC * CHUNK :
                                  q * QC * CHUNK + npiece],
                            obs[q][:, :npiece],
                        )

    nc.compile()
    return nc


_NC_CACHE = None


def _get_nc():
    global _NC_CACHE
    if _NC_CACHE is None:
        _NC_CACHE = _build_bass()
    return _NC_CACHE


def _F64(x, y):
    return (
        3.0 * (1.0 - x) ** 2 * np.exp(-(x**2) - (y + 1.0) ** 2)
        - 10.0 * (x / 5.0 - x**3 - y**5) * np.exp(-(x**2) - y**2)
        - 1.0 / (3.0 ** np.exp(-((x + 1.0) ** 2) - y**2))
    )


def kernel(data, embed, W1, b1, W2, b2, W3, b3):
    data = np.asarray(data)
    table = np.asarray(embed, dtype=np.float32).reshape(-1)
    W1 = np.ascontiguousarray(W1, dtype=np.float32)
    W2 = np.ascontiguousarray(W2, dtype=np.float32)
    W3 = np.ascontiguousarray(W3, dtype=np.float32)
    b1c = np.ascontiguousarray(b1, dtype=np.float32).reshape(H1, 1)
    b2c = np.ascontiguousarray(b2, dtype=np.float32).reshape(H2, 1)
    b3c = np.asarray(b3, dtype=np.float32).reshape(NCLS)

    nc = _get_nc()

    npairs = (CHUNKS + 1) // 2
    in_maps = []
    for c in range(NCORES):
        dshard = data[c * NPC_RAW : (c + 1) * NPC_RAW]
        dpad = np.zeros((npairs * 2 * CHUNK, D), dtype=dshard.dtype)
        dpad[:NPC_RAW] = dshard
        # fused gather + tile-blocked transpose: [npairs, D(feat), 2*CHUNK]
        xt = np.ascontiguousarray(
            table[dpad.reshape(npairs, 2 * CHUNK, D).transpose(0, 2, 1)]
        )
        in_maps.append(
            {"x_t": xt, "w1": W1, "w2": W2, "w3": W3, "b1": b1c, "b2": b2c}
        )

    res = run_bass_kernel_spmd(nc, in_maps, core_ids=list(range(NCORES)))

    outs = []
    for c in range(NCORES):
        op = res.results[c]["out_pairs"]  # [2, NPC]
        outs.append(op.T[:NPC_RAW])
    out_all = np.concatenate(outs, axis=0) + b3c  # [N, 2] fp32

    x64 = out_all[:, 0].astype(np.float64)
    y64 = out_all[:, 1].astype(np.float64)
    pred = _F64(x64, y64)
    idx = int(np.argmax(pred))
    return out_all[idx].astype(np.float32)



# revision 3
# speedup vs baseline: 1.4158x; 1.4158x over previous
"""Trainium2 Bass kernel for nn_NeuralNet_19516331393457 (dense_mlp).

Pipeline: x = embed[data] (48-entry table); h1 = relu(x@W1+b1);
h2 = tanh(h1@W2+b2); out = h2@W3+b3; return out[argmax(F(out0, out1))].

Strategy (data-parallel over N=500000 on 8 cores), fp16 on device:
  - Host: tiny-table gather embed[data] in fp16 fused with a tile-blocked
    transpose so each device quad-tile is a contiguous [128 feat, 2048
    samples] fp16 block (halves DMA traffic vs fp32).
  - Device (per core, 63488 padded samples = 31 quads of 4x512 chunks):
      * MM1 per 512-chunk into 2-bank [128,1024] fp32 PSUM pair tiles;
        relu (+b1) evicts 1024 cols per op, engine split DVE/ACT
      * MM2 quad-packed: chunks stacked 2-up in partitions (dup W2
        stationary at PE col groups 0/64) into a [128,1024] 2-bank PSUM
        tile; ONE tanh (+b2 stacked) evicts 4 chunks
      * MM3 pair-packed: stacked h2 [128,512] x block-diagonal W3 slab ->
        [4,512] at PSUM partition base {0,32,64} (PE quadrant rule), so
        one PSUM bank collects 3 pairs; 21 copies + 21 output DMAs
  - Host: decode packed outs, compute F in float64, global argmax,
    return out[argmax] + b3.
"""

import numpy as np

import concourse.mybir as mybir
import concourse.tile as tile
from concourse import bacc
from concourse.bass_utils import run_bass_kernel_spmd

N = 500000
D = 128
H1 = 128
H2 = 64
NCLS = 2
NCORES = 8
CHUNK = 512
NPC_RAW = N // NCORES              # 62500 samples per core
QUADS = 31                         # quads of 4 chunks per core
PAIRS = 2 * QUADS                  # 62 pairs
CHUNKS = 4 * QUADS                 # 124 chunks
NPC = CHUNKS * CHUNK               # 63488 padded samples per core
GROUPS = -(-PAIRS // 3)            # 21 po groups (3 pairs per PSUM bank)

_F16 = mybir.dt.float16
_F32 = mybir.dt.float32


def _build_bass():
    nc = bacc.Bacc(
        "TRN2",
        target_bir_lowering=False,
        debug=False,
        enable_asserts=False,
        num_devices=NCORES,
    )
    x_t = nc.dram_tensor("x_t", [QUADS, D, 4 * CHUNK], _F16,
                         kind="ExternalInput")
    w1 = nc.dram_tensor("w1", [D, H1], _F16, kind="ExternalInput")
    w2d = nc.dram_tensor("w2d", [H1, 2 * H2], _F16, kind="ExternalInput")
    w3b = nc.dram_tensor("w3b", [2 * H2, 128], _F16, kind="ExternalInput")
    b1 = nc.dram_tensor("b1", [H1, 1], _F32, kind="ExternalInput")
    b2s = nc.dram_tensor("b2s", [2 * H2, 1], _F32, kind="ExternalInput")
    out_d = nc.dram_tensor("out_d", [GROUPS, 68, CHUNK], _F32,
                           kind="ExternalOutput")

    with tile.TileContext(nc) as tc:
        with (
            tc.tile_pool(name="w", bufs=1) as wpool,
            tc.tile_pool(name="x", bufs=3) as xpool,
            tc.tile_pool(name="h1", bufs=3) as h1pool,
            tc.tile_pool(name="h2", bufs=2) as h2pool,
            tc.tile_pool(name="ob", bufs=2) as obpool,
            tc.tile_pool(name="p1", bufs=2, space="PSUM") as p1pool,
            tc.tile_pool(name="p2", bufs=1, space="PSUM") as p2pool,
            tc.tile_pool(name="po", bufs=2, space="PSUM") as popool,
        ):
            w1sb = wpool.tile([D, H1], _F16)
            nc.sync.dma_start(w1sb[:], w1[:, :])
            w2sb = wpool.tile([H1, 2 * H2], _F16)
            nc.sync.dma_start(w2sb[:], w2d[:, :])
            w3sb = wpool.tile([2 * H2, 128], _F16)
            nc.sync.dma_start(w3sb[:], w3b[:, :])
            b1sb = wpool.tile([H1, 1], _F32)
            nc.sync.dma_start(b1sb[:], b1[:, :])
            b2sb = wpool.tile([2 * H2, 1], _F32)
            nc.sync.dma_start(b2sb[:], b2s[:, :])

            pos = {}   # po group -> psum tile [128, CHUNK]
            relu_on_act = 0

            for q in range(QUADS):
                xt = xpool.tile([D, 4 * CHUNK], _F16, name=f"xt{q}",
                                tag="xt")
                nc.sync.dma_start(xt[:], x_t[q, :, :])

                h1s = []
                for half in range(2):  # pair A (chunks 0,1), pair B (2,3)
                    p1 = p1pool.tile([H1, 2 * CHUNK], _F32,
                                     name=f"p1_{q}_{half}", tag="p1")
                    base = half * 2 * CHUNK
                    nc.tensor.matmul(p1[:, 0:CHUNK], w1sb[:],
                                     xt[:, base:base + CHUNK],
                                     start=True, stop=True)
                    nc.tensor.matmul(p1[:, CHUNK:2 * CHUNK], w1sb[:],
                                     xt[:, base + CHUNK:base + 2 * CHUNK],
                                     start=True, stop=True)
                    h1t = h1pool.tile([H1, 2 * CHUNK], _F16,
                                      name=f"h1_{q}_{half}", tag="h1")
                    p_idx = 2 * q + half
                    if (p_idx % 7) < 3:   # ~43% of relus on ACT
                        nc.scalar.activation(
                            h1t[:], p1[:],
                            mybir.ActivationFunctionType.Relu, bias=b1sb[:],
                        )
                    else:
                        nc.vector.tensor_scalar(
                            h1t[:], p1[:], b1sb[:], 0.0,
                            mybir.AluOpType.add, mybir.AluOpType.max,
                        )
                    h1s.append(h1t)

                # MM2 quad: 4 chunks into one [128,1024] 2-bank tile.
                # chunk (half,0) -> rows 0:64, chunk (half,1) -> rows
                # 64:128, pair half -> cols half*512.  Ordered to batch
                # the two stationaries.
                p2 = p2pool.tile([128, 2 * CHUNK], _F32, name=f"p2_{q}",
                                 tag="p2")
                for half in range(2):
                    nc.tensor.matmul(
                        p2[0:H2, half * CHUNK:(half + 1) * CHUNK],
                        w2sb[:, 0:H2], h1s[half][:, 0:CHUNK],
                        start=True, stop=True)
                for half in range(2):
                    nc.tensor.matmul(
                        p2[H2:2 * H2, half * CHUNK:(half + 1) * CHUNK],
                        w2sb[:, H2:2 * H2], h1s[half][:, CHUNK:2 * CHUNK],
                        start=True, stop=True)

                h2t = h2pool.tile([128, 2 * CHUNK], _F16, name=f"h2_{q}",
                                  tag="h2")
                nc.scalar.activation(
                    h2t[:], p2[:], mybir.ActivationFunctionType.Tanh,
                    bias=b2sb[:],
                )

                for half in range(2):
                    p_idx = 2 * q + half
                    g, m = divmod(p_idx, 3)
                    if m == 0:
                        pos[g] = popool.tile([128, CHUNK], _F32,
                                             name=f"po{g}", tag="po")
                    nc.tensor.matmul(
                        pos[g][32 * m:32 * m + 4, :],
                        w3sb[:, 32 * m:32 * m + 4],
                        h2t[:, half * CHUNK:(half + 1) * CHUNK],
                        start=True, stop=True)
                    if m == 2 or p_idx == PAIRS - 1:
                        ob = obpool.tile([68, CHUNK], _F32, name=f"ob{g}",
                                         tag="ob")
                        nc.vector.tensor_copy(ob[:], pos[g][0:68, :])
                        nc.sync.dma_start(out_d[g, :, :], ob[:])

    nc.compile()
    return nc


_NC_CACHE = None


def _get_nc():
    global _NC_CACHE
    if _NC_CACHE is None:
        _NC_CACHE = _build_bass()
    return _NC_CACHE


def _weight_tensors(W1, b1, W2, b2, W3):
    w1 = np.ascontiguousarray(W1, dtype=np.float16)
    w2dm = np.concatenate([W2, W2], axis=1).astype(np.float16)
    # w3b[:, 4j+r]: r in {0,1} -> rows 0:64 = W3[:, r]; r in {2,3} ->
    # rows 64:128 = W3[:, r-2]; zero elsewhere.  Identical for every j,
    # so any aligned 4-col slice carries the pair block.
    w3bm = np.zeros((2 * H2, 128), dtype=np.float16)
    for r in range(2):
        w3bm[0:H2, r::4] = W3[:, r:r + 1].astype(np.float16)
        w3bm[H2:2 * H2, r + 2::4] = W3[:, r:r + 1].astype(np.float16)
    b1c = np.ascontiguousarray(b1, dtype=np.float32).reshape(H1, 1)
    b2sc = np.concatenate([b2, b2]).astype(np.float32).reshape(2 * H2, 1)
    return {"w1": w1, "w2d": np.ascontiguousarray(w2dm),
            "w3b": np.ascontiguousarray(w3bm), "b1": b1c, "b2s": b2sc}


def _core_inmap(data, table16, core, weights):
    dshard = data[core * NPC_RAW:(core + 1) * NPC_RAW]
    dpad = np.zeros((NPC, D), dtype=dshard.dtype)
    dpad[:NPC_RAW] = dshard
    # fused fp16 gather + tile-blocked transpose: [QUADS, D, 2048]
    xt = np.ascontiguousarray(
        table16[dpad.reshape(QUADS, 4 * CHUNK, D).transpose(0, 2, 1)]
    )
    return {"x_t": xt, **weights}


def _decode_core(arr):
    """[GROUPS, 68, CHUNK] fp32 packed outs -> [NPC, 2] fp32."""
    out = np.empty((NPC, 2), dtype=np.float32)
    for g in range(GROUPS):
        for m in range(3):
            p = 3 * g + m
            if p >= PAIRS:
                break
            blk = arr[g, 32 * m:32 * m + 4, :]     # [4, CHUNK]
            s0 = p * 2 * CHUNK
            out[s0:s0 + CHUNK, 0] = blk[0]
            out[s0:s0 + CHUNK, 1] = blk[1]
            out[s0 + CHUNK:s0 + 2 * CHUNK, 0] = blk[2]
            out[s0 + CHUNK:s0 + 2 * CHUNK, 1] = blk[3]
    return out


def _F64(x, y):
    return (
        3.0 * (1.0 - x) ** 2 * np.exp(-(x**2) - (y + 1.0) ** 2)
        - 10.0 * (x / 5.0 - x**3 - y**5) * np.exp(-(x**2) - y**2)
        - 1.0 / (3.0 ** np.exp(-((x + 1.0) ** 2) - y**2))
    )


def kernel(data, embed, W1, b1, W2, b2, W3, b3):
    data = np.asarray(data)
    table16 = np.asarray(embed, dtype=np.float32).reshape(-1).astype(
        np.float16)
    b3c = np.asarray(b3, dtype=np.float32).reshape(NCLS)

    nc = _get_nc()
    weights = _weight_tensors(W1, b1, W2, b2, W3)
    in_maps = [_core_inmap(data, table16, c, weights) for c in range(NCORES)]

    res = run_bass_kernel_spmd(nc, in_maps, core_ids=list(range(NCORES)))

    outs = []
    for c in range(NCORES):
        outs.append(_decode_core(res.results[c]["out_d"])[:NPC_RAW])
    out_all = np.concatenate(outs, axis=0) + b3c  # [N, 2] fp32

    x64 = out_all[:, 0].astype(np.float64)
    y64 = out_all[:, 1].astype(np.float64)
    pred = _F64(x64, y64)
    idx = int(np.argmax(pred))
    return out_all[idx].astype(np.float32)


# revision 4
# speedup vs baseline: 1.6590x; 1.1718x over previous
"""Trainium2 Bass kernel for nn_NeuralNet_19516331393457 (dense_mlp).

Pipeline: x = embed[data] (48-entry table); h1 = relu(x@W1+b1);
h2 = tanh(h1@W2+b2); out = h2@W3+b3; return out[argmax(F(out0, out1))].

Strategy (data-parallel over N=500000 on 8 cores), fp16 on device:
  - Host: tiny-table gather embed[data] in fp16 fused with a tile-blocked
    transpose so each device pair-tile is a contiguous [128 feat, 1024
    samples] fp16 block (halves DMA traffic vs fp32).
  - Device (per core, 63488 padded samples = 62 pairs of 2x512 chunks),
    software-pipelined with a 2-stage skew so the in-order PE queue never
    waits on evictions issued the same iteration:
      iter p: DMA x(p); MM1(p) x2; MM2(p-1) x2; MM3(p-2); relu(p) x2;
              tanh(p-1); po-bank copy + out DMA when a group completes
      * MM2 pair-packed: chunk A -> PSUM rows 0:64 (dup W2 stationary at
        PE col group 0), chunk B -> rows 64:128 (col group 64); one tanh
        (+b2 stacked) evicts both chunks
      * MM3 pair-packed: stacked h2 [128,512] x block-diagonal W3 slab ->
        [4,512] at PSUM partition base {0,32,64} (PE quadrant rule), so
        one PSUM bank collects 3 pairs; 21 copies + 21 output DMAs
      * relu/copies statically load-balanced between DVE and ACT (tanh is
        ACT-only); all eviction reads are single-PSUM-bank 512-col ops
  - Host: decode packed outs, compute F in float64, global argmax,
    return out[argmax] + b3.
"""

import numpy as np

import concourse.mybir as mybir
import concourse.tile as tile
from concourse import bacc
from concourse.bass_utils import run_bass_kernel_spmd

N = 500000
D = 128
H1 = 128
H2 = 64
NCLS = 2
NCORES = 8
CHUNK = 512
NPC_RAW = N // NCORES              # 62500 samples per core
PAIRS = 62                         # pairs of 2 chunks per core
CHUNKS = 2 * PAIRS                 # 124 chunks
NPC = CHUNKS * CHUNK               # 63488 padded samples per core
GROUPS = -(-PAIRS // 3)            # 21 po groups (3 pairs per PSUM bank)

_F16 = mybir.dt.float16
_F32 = mybir.dt.float32

# measured per-op eviction costs (ns) for static DVE/ACT load balancing
_DVE_RELU, _ACT_RELU = 669.0, 638.0
_DVE_COPY, _ACT_COPY = 744.0, 700.0
_ACT_TANH = 638.0


def _build_bass():
    nc = bacc.Bacc(
        "TRN2",
        target_bir_lowering=False,
        debug=False,
        enable_asserts=False,
        num_devices=NCORES,
    )
    x_t = nc.dram_tensor("x_t", [PAIRS, D, 2 * CHUNK], _F16,
                         kind="ExternalInput")
    w1 = nc.dram_tensor("w1", [D, H1], _F16, kind="ExternalInput")
    w2d = nc.dram_tensor("w2d", [H1, 2 * H2], _F16, kind="ExternalInput")
    w3b = nc.dram_tensor("w3b", [2 * H2, 128], _F16, kind="ExternalInput")
    b1 = nc.dram_tensor("b1", [H1, 1], _F32, kind="ExternalInput")
    b2s = nc.dram_tensor("b2s", [2 * H2, 1], _F32, kind="ExternalInput")
    out_d = nc.dram_tensor("out_d", [GROUPS, 68, CHUNK], _F32,
                           kind="ExternalOutput")

    load = {"dve": 0.0, "act": 0.0}

    with tile.TileContext(nc) as tc:
        with (
            tc.tile_pool(name="w", bufs=1) as wpool,
            tc.tile_pool(name="x", bufs=4) as xpool,
            tc.tile_pool(name="h1", bufs=4) as h1pool,
            tc.tile_pool(name="h2", bufs=3) as h2pool,
            tc.tile_pool(name="ob", bufs=2) as obpool,
            tc.tile_pool(name="p1", bufs=4, space="PSUM") as p1pool,
            tc.tile_pool(name="p2", bufs=2, space="PSUM") as p2pool,
            tc.tile_pool(name="po", bufs=2, space="PSUM") as popool,
        ):
            w1sb = wpool.tile([D, H1], _F16)
            nc.sync.dma_start(w1sb[:], w1[:, :])
            w2sb = wpool.tile([H1, 2 * H2], _F16)
            nc.sync.dma_start(w2sb[:], w2d[:, :])
            w3sb = wpool.tile([2 * H2, 128], _F16)
            nc.sync.dma_start(w3sb[:], w3b[:, :])
            b1sb = wpool.tile([H1, 1], _F32)
            nc.sync.dma_start(b1sb[:], b1[:, :])
            b2sb = wpool.tile([2 * H2, 1], _F32)
            nc.sync.dma_start(b2sb[:], b2s[:, :])

            # prefetch the ACT table set (relu/tanh share one) under the
            # first input DMAs
            warm = wpool.tile([H1, 1], _F32)
            nc.scalar.activation(warm[:], b1sb[:],
                                 mybir.ActivationFunctionType.Relu)

            xts = {}     # pair -> [128, 1024] f16
            p1s = {}     # chunk -> [128, 512] f32 psum
            h1s = {}     # chunk -> [128, 512] f16
            p2s = {}     # pair -> [128, 512] f32 psum (A rows 0:64, B 64:128)
            h2s = {}     # pair -> [128, 512] f16
            pos = {}     # group -> [128, 512] f32 psum

            def emit_dma_mm1(p):
                xt = xpool.tile([D, 2 * CHUNK], _F16, name=f"xt{p}",
                                tag="xt")
                nc.sync.dma_start(xt[:], x_t[p, :, :])
                xts[p] = xt
                for half in range(2):
                    c = 2 * p + half
                    p1 = p1pool.tile([H1, CHUNK], _F32, name=f"p1_{c}",
                                     tag="p1")
                    nc.tensor.matmul(
                        p1[:], w1sb[:],
                        xt[:, half * CHUNK:(half + 1) * CHUNK],
                        start=True, stop=True)
                    p1s[c] = p1

            def emit_relu(p):
                for half in range(2):
                    c = 2 * p + half
                    h1t = h1pool.tile([H1, CHUNK], _F16, name=f"h1_{c}",
                                      tag="h1")
                    if load["act"] + _ACT_RELU <= load["dve"] + _DVE_RELU:
                        load["act"] += _ACT_RELU
                        nc.scalar.activation(
                            h1t[:], p1s[c][:],
                            mybir.ActivationFunctionType.Relu, bias=b1sb[:])
                    else:
                        load["dve"] += _DVE_RELU
                        nc.vector.tensor_scalar(
                            h1t[:], p1s[c][:], b1sb[:], 0.0,
                            mybir.AluOpType.add, mybir.AluOpType.max)
                    h1s[c] = h1t
                    del p1s[c]

            def emit_mm2(p):
                p2 = p2pool.tile([128, CHUNK], _F32, name=f"p2_{p}",
                                 tag="p2")
                nc.tensor.matmul(p2[0:H2, :], w2sb[:, 0:H2],
                                 h1s[2 * p][:], start=True, stop=True)
                nc.tensor.matmul(p2[H2:2 * H2, :], w2sb[:, H2:2 * H2],
                                 h1s[2 * p + 1][:], start=True, stop=True)
                p2s[p] = p2
                del h1s[2 * p], h1s[2 * p + 1]

            def emit_tanh(p):
                h2t = h2pool.tile([128, CHUNK], _F16, name=f"h2_{p}",
                                  tag="h2")
                load["act"] += _ACT_TANH
                nc.scalar.activation(
                    h2t[:], p2s[p][:], mybir.ActivationFunctionType.Tanh,
                    bias=b2sb[:])
                h2s[p] = h2t
                del p2s[p]

            def emit_mm3(p):
                g, m = divmod(p, 3)
                if m == 0:
                    pos[g] = popool.tile([128, CHUNK], _F32,
                                         name=f"po{g}", tag="po")
                nc.tensor.matmul(
                    pos[g][32 * m:32 * m + 4, :],
                    w3sb[:, 32 * m:32 * m + 4], h2s[p][:],
                    start=True, stop=True)
                del h2s[p]
                if m == 2 or p == PAIRS - 1:
                    ob = obpool.tile([68, CHUNK], _F32, name=f"ob{g}",
                                     tag="ob")
                    if load["act"] + _ACT_COPY <= load["dve"] + _DVE_COPY:
                        load["act"] += _ACT_COPY
                        nc.scalar.copy(ob[:], pos[g][0:68, :])
                    else:
                        load["dve"] += _DVE_COPY
                        nc.vector.tensor_copy(ob[:], pos[g][0:68, :])
                    nc.sync.dma_start(out_d[g, :, :], ob[:])
                    del pos[g]

            for p in range(PAIRS + 2):
                if p < PAIRS:
                    emit_dma_mm1(p)
                if 0 <= p - 1 < PAIRS:
                    emit_mm2(p - 1)
                if 0 <= p - 2 < PAIRS:
                    emit_mm3(p - 2)
                if p < PAIRS:
                    emit_relu(p)
                if 0 <= p - 1 < PAIRS:
                    emit_tanh(p - 1)

    nc.compile()
    return nc


_NC_CACHE = None


def _get_nc():
    global _NC_CACHE
    if _NC_CACHE is None:
        _NC_CACHE = _build_bass()
    return _NC_CACHE


def _weight_tensors(W1, b1, W2, b2, W3):
    w1 = np.ascontiguousarray(W1, dtype=np.float16)
    w2dm = np.concatenate([W2, W2], axis=1).astype(np.float16)
    # w3b[:, 4j+r]: r in {0,1} -> rows 0:64 = W3[:, r]; r in {2,3} ->
    # rows 64:128 = W3[:, r-2]; zero elsewhere.  Identical for every j,
    # so any aligned 4-col slice carries the pair block.
    w3bm = np.zeros((2 * H2, 128), dtype=np.float16)
    for r in range(2):
        w3bm[0:H2, r::4] = W3[:, r:r + 1].astype(np.float16)
        w3bm[H2:2 * H2, r + 2::4] = W3[:, r:r + 1].astype(np.float16)
    b1c = np.ascontiguousarray(b1, dtype=np.float32).reshape(H1, 1)
    b2sc = np.concatenate([b2, b2]).astype(np.float32).reshape(2 * H2, 1)
    return {"w1": w1, "w2d": np.ascontiguousarray(w2dm),
            "w3b": np.ascontiguousarray(w3bm), "b1": b1c, "b2s": b2sc}


def _core_inmap(data, table16, core, weights):
    dshard = data[core * NPC_RAW:(core + 1) * NPC_RAW]
    dpad = np.zeros((NPC, D), dtype=dshard.dtype)
    dpad[:NPC_RAW] = dshard
    # fused fp16 gather + tile-blocked transpose: [PAIRS, D, 1024]
    xt = np.ascontiguousarray(
        table16[dpad.reshape(PAIRS, 2 * CHUNK, D).transpose(0, 2, 1)]
    )
    return {"x_t": xt, **weights}


def _decode_core(arr):
    """[GROUPS, 68, CHUNK] fp32 packed outs -> [NPC, 2] fp32."""
    out = np.empty((NPC, 2), dtype=np.float32)
    for g in range(GROUPS):
        for m in range(3):
            p = 3 * g + m
            if p >= PAIRS:
                break
            blk = arr[g, 32 * m:32 * m + 4, :]     # [4, CHUNK]
            s0 = p * 2 * CHUNK
            out[s0:s0 + CHUNK, 0] = blk[0]
            out[s0:s0 + CHUNK, 1] = blk[1]
            out[s0 + CHUNK:s0 + 2 * CHUNK, 0] = blk[2]
            out[s0 + CHUNK:s0 + 2 * CHUNK, 1] = blk[3]
    return out


def _F64(x, y):
    return (
        3.0 * (1.0 - x) ** 2 * np.exp(-(x**2) - (y + 1.0) ** 2)
        - 10.0 * (x / 5.0 - x**3 - y**5) * np.exp(-(x**2) - y**2)
        - 1.0 / (3.0 ** np.exp(-((x + 1.0) ** 2) - y**2))
    )


def kernel(data, embed, W1, b1, W2, b2, W3, b3):
    data = np.asarray(data)
    table16 = np.asarray(embed, dtype=np.float32).reshape(-1).astype(
        np.float16)
    b3c = np.asarray(b3, dtype=np.float32).reshape(NCLS)

    nc = _get_nc()
    weights = _weight_tensors(W1, b1, W2, b2, W3)
    in_maps = [_core_inmap(data, table16, c, weights) for c in range(NCORES)]

    res = run_bass_kernel_spmd(nc, in_maps, core_ids=list(range(NCORES)))

    outs = []
    for c in range(NCORES):
        outs.append(_decode_core(res.results[c]["out_d"])[:NPC_RAW])
    out_all = np.concatenate(outs, axis=0) + b3c  # [N, 2] fp32

    x64 = out_all[:, 0].astype(np.float64)
    y64 = out_all[:, 1].astype(np.float64)
    pred = _F64(x64, y64)
    idx = int(np.argmax(pred))
    return out_all[idx].astype(np.float32)
